# revision 13
# baseline (speedup 1.0000x reference)
"""Trainium2 Bass kernel: causal multi-head attention with RoPE + output proj.

Reference computation (B=2, T=2048, D=2048, NH=16, DH=128, fp32):
    qkv = x @ w_qkv.T -> (B,T,3,NH,DH); rope(q,k); causal softmax attention;
    out = attn @ v; y = out @ w_op.T

Sharding over 8 NeuronCores: data parallel on batch (2) x tensor parallel on
heads (4 head-groups of 4 heads). Each core computes q/k/v for its 4 heads
and their attention output out^T (a 512-row d-block of out^T). A per-head
8-core AllToAll exchanges out^T t-slices so that core (b, g) ends up with the
FULL out^T for t-slice g of batch b; it then computes those 512 rows of the
final projection y = out @ w_op.T against the full w_op -- no inter-core
reduction needed. The host concatenates the 8 row-shards.

Batch routing through the single 8-core AllToAll is data-driven: each source
writes its chunk to both paired slots (r, r+4), and each destination combines
slot g and g+4 with a host-supplied 0/1 select (its own batch), keeping the
program SPMD-identical across cores.

All matmuls run as float32r (fp32 storage, FP22 multiply, full PE rate).
Scores are computed transposed (St[tk,tq] = k^T q) so the attn@v matmul needs
no on-chip transposes; the softmax denominator comes from a ones-vector
matmul accumulated alongside attn@v in PSUM.
"""

import os
import sys

import numpy as np

# The Bass program executes through jax/PJRT on the axon-tunneled NeuronCores.
# Guard against an environment that pinned jax to CPU before we import it.
if os.environ.get("TRN_TERMINAL_POOL_IPS") and "jax" not in sys.modules:
    if os.environ.get("JAX_PLATFORMS", "").strip() in ("cpu",):
        os.environ["JAX_PLATFORMS"] = ""

if "/opt/trn_rl_repo" not in sys.path and os.path.isdir("/opt/trn_rl_repo"):
    sys.path.append("/opt/trn_rl_repo")

import concourse.bacc as bacc
import concourse.mybir as mybir
import concourse.tile as tile
from concourse.bass_utils import run_bass_kernel_spmd

# Problem constants (hardcoded per contract).
B, T, D = 2, 2048, 2048
NH, DH = 16, 128
ROPE_THETA = 10000.0
N_CORES = 8
HPG = 4          # heads per core (head group size)
KD = D // 128    # 16 d-chunks of 128
TB = T // 512    # 4 t-blocks of 512
TC = T // 128    # 16 t-chunks of 128
SCALE = 1.0 / float(np.sqrt(DH))

F32 = mybir.dt.float32
FR = mybir.dt.float32r

_CACHE = {}


def _build_program():
    nc = bacc.Bacc("TRN2", target_bir_lowering=False, debug=False,
                   num_devices=N_CORES)

    # ---- external I/O (per-core shards; same program on all 8 cores) ----
    xt_d = nc.dram_tensor("xt", [128, KD, T], FR, kind="ExternalInput")
    wqk_d = nc.dram_tensor("wqk", [128, KD, 8, 128], FR, kind="ExternalInput")
    wv_d = nc.dram_tensor("wv", [128, KD, 512], FR, kind="ExternalInput")
    wop_d = nc.dram_tensor("wop", [128, KD, D], FR, kind="ExternalInput")
    cos_d = nc.dram_tensor("cosT", [128, T], F32, kind="ExternalInput")
    sin_d = nc.dram_tensor("sinT", [128, T], F32, kind="ExternalInput")
    mask_d = nc.dram_tensor("masks", [128, 4, 512], F32, kind="ExternalInput")
    ones_d = nc.dram_tensor("ones", [128, 1], FR, kind="ExternalInput")
    sel_d = nc.dram_tensor("sel", [128, 2], F32, kind="ExternalInput")
    y_d = nc.dram_tensor("y", [512, D], F32, kind="ExternalOutput")

    groups = [[0, 1, 2, 3, 4, 5, 6, 7]]

    with tile.TileContext(nc) as tc:
        with tc.tile_pool(name="dram", bufs=1, space="DRAM") as dram:
            qk_spill = dram.tile([2, HPG, 128, T], F32)   # q/k pre-RoPE
            v_spill = dram.tile([TC, 128, 512], FR)

            # ------------- phase 1: q/k/v projections (single pass) -------
            with tc.tile_pool(name="w1", bufs=1) as w1, \
                 tc.tile_pool(name="xs", bufs=2) as xs, \
                 tc.tile_pool(name="st1", bufs=3) as st1, \
                 tc.tile_pool(name="ps1", bufs=4, space="PSUM") as ps1:
                wqk_sb = w1.tile([128, KD, 8, 128], FR, name="wqk_sb")
                nc.sync.dma_start(wqk_sb[:], wqk_d.ap())
                wv_sb = w1.tile([128, KD, 512], FR, name="wv_sb")
                nc.sync.dma_start(wv_sb[:], wv_d.ap())
                for tb in range(TB):
                    xt = xs.tile([128, KD, 512], FR, name=f"xt{tb}", tag="xt")
                    nc.sync.dma_start(xt[:], xt_d.ap()[:, :, tb * 512:(tb + 1) * 512])
                    for j in range(8):  # q heads 0-3, k heads 4-7
                        ps = ps1.tile([128, 512], F32, name=f"psqk{tb}_{j}", tag="ps")
                        for kd in range(KD):
                            nc.tensor.matmul(ps[:], wqk_sb[:, kd, j, :], xt[:, kd, :],
                                             start=(kd == 0), stop=(kd == KD - 1))
                        st = st1.tile([128, 512], F32, name=f"stqk{tb}_{j}", tag="st")
                        nc.vector.tensor_copy(st[:], ps[:])
                        nc.sync.dma_start(
                            qk_spill[j // HPG, j % HPG, :, tb * 512:(tb + 1) * 512],
                            st[:])
                    for i in range(4):  # v t-chunks
                        tci = tb * 4 + i
                        ps = ps1.tile([128, 512], F32, name=f"psv{tb}_{i}", tag="ps")
                        for kd in range(KD):
                            nc.tensor.matmul(ps[:], xt[:, kd, i * 128:(i + 1) * 128],
                                             wv_sb[:, kd, :],
                                             start=(kd == 0), stop=(kd == KD - 1))
                        stv = st1.tile([128, 512], FR, name=f"stv{tb}_{i}", tag="stv")
                        nc.vector.tensor_copy(stv[:], ps[:])
                        nc.sync.dma_start(v_spill[tci], stv[:])

            # ------------- phase 2: attention + per-head AllToAll ---------
            a2a_in = [dram.tile([8, 128, 512], FR, name=f"a2ain{h}", tag=f"a2ain{h}")
                      for h in range(HPG)]
            a2a_out = [dram.tile([8, 128, 512], FR, name=f"a2aout{h}",
                                 tag=f"a2aout{h}") for h in range(HPG)]
            with tc.tile_pool(name="vres", bufs=1) as vres, \
                 tc.tile_pool(name="consts", bufs=1) as consts, \
                 tc.tile_pool(name="qk", bufs=2) as qkp, \
                 tc.tile_pool(name="rot", bufs=2) as rotp, \
                 tc.tile_pool(name="ropeqk", bufs=2) as ropep, \
                 tc.tile_pool(name="ptile", bufs=6) as ptp, \
                 tc.tile_pool(name="zt", bufs=2) as ztp, \
                 tc.tile_pool(name="och", bufs=3) as ochp, \
                 tc.tile_pool(name="sps", bufs=3, space="PSUM") as sps, \
                 tc.tile_pool(name="ops", bufs=2, space="PSUM") as ops, \
                 tc.tile_pool(name="zps", bufs=2, space="PSUM") as zps:
                v_sb = vres.tile([128, TC, 512], FR)
                nc.sync.dma_start(
                    v_sb[:], v_spill[:].rearrange("c p e -> p c e"))
                cos_sb = consts.tile([128, T], F32)
                sin_sb = consts.tile([128, T], F32)
                mask_sb = consts.tile([128, 4, 512], F32)
                ones_sb = consts.tile([128, 1], FR)
                nc.sync.dma_start(cos_sb[:], cos_d.ap())
                nc.sync.dma_start(sin_sb[:], sin_d.ap())
                nc.sync.dma_start(mask_sb[:], mask_d.ap())
                nc.sync.dma_start(ones_sb[:], ones_d.ap())
                for h in range(HPG):
                    qraw = qkp.tile([128, T], F32, name=f"qraw{h}", tag="qraw")
                    nc.sync.dma_start(qraw[:], qk_spill[0, h])
                    kraw = qkp.tile([128, T], F32, name=f"kraw{h}", tag="kraw")
                    nc.sync.dma_start(kraw[:], qk_spill[1, h])
                    # rotate_half via partition-swapped reload (DVE ops cannot
                    # cross partitions); the sign lives in the sin table.
                    qrot = rotp.tile([128, T], F32, name=f"qrot{h}", tag="qrot")
                    nc.sync.dma_start(qrot[0:64, :], qk_spill[0, h, 64:128, :])
                    nc.sync.dma_start(qrot[64:128, :], qk_spill[0, h, 0:64, :])
                    krot = rotp.tile([128, T], F32, name=f"krot{h}", tag="krot")
                    nc.sync.dma_start(krot[0:64, :], qk_spill[1, h, 64:128, :])
                    nc.sync.dma_start(krot[64:128, :], qk_spill[1, h, 0:64, :])

                    # RoPE: out = a*cos + rotate_half(a)*sin_signed
                    qr = ropep.tile([128, T], FR, name=f"qr{h}", tag="qr")
                    kr = ropep.tile([128, T], FR, name=f"kr{h}", tag="kr")
                    for src, rot, dst, nm in ((qraw, qrot, qr, "q"),
                                              (kraw, krot, kr, "k")):
                        nc.vector.tensor_mul(src[:], src[:], cos_sb[:])
                        nc.vector.tensor_mul(rot[:], rot[:], sin_sb[:])
                        nc.vector.tensor_add(dst[:], src[:], rot[:])

                    for i2 in range(4):  # tq blocks of 512
                        n_j = 4 * i2 + 4
                        o_ps = ops.tile([128, 512], F32, name=f"o{h}_{i2}", tag="o")
                        z_ps = zps.tile([1, 512], F32, name=f"z{h}_{i2}", tag="z")
                        for j2 in range(n_j):  # tk chunks of 128
                            s_ps = sps.tile([128, 512], F32,
                                            name=f"s{h}_{i2}_{j2}", tag="s")
                            nc.tensor.matmul(s_ps[:], kr[:, j2 * 128:(j2 + 1) * 128],
                                             qr[:, i2 * 512:(i2 + 1) * 512],
                                             start=True, stop=True)
                            pt = ptp.tile([128, 512], FR,
                                          name=f"p{h}_{i2}_{j2}", tag="p")
                            nc.scalar.activation(
                                pt[:], s_ps[:],
                                mybir.ActivationFunctionType.Exp, scale=SCALE)
                            m = j2 - 4 * i2
                            if m >= 0:  # partial (diagonal) tile: apply mask
                                nc.vector.tensor_mul(pt[:], pt[:], mask_sb[:, m, :])
                            nc.tensor.matmul(o_ps[:], v_sb[:, j2, h * 128:(h + 1) * 128],
                                             pt[:], start=(j2 == 0),
                                             stop=(j2 == n_j - 1))
                            nc.tensor.matmul(z_ps[:], ones_sb[:], pt[:],
                                             start=(j2 == 0), stop=(j2 == n_j - 1))
                        zrec = ztp.tile([1, 512], F32, name=f"zr{h}_{i2}", tag="zr")
                        nc.vector.reciprocal(zrec[:], z_ps[:])
                        zb = ztp.tile([128, 512], F32, name=f"zb{h}_{i2}", tag="zb")
                        nc.gpsimd.partition_broadcast(zb[:], zrec[:])
                        och = ochp.tile([128, 512], FR, name=f"oc{h}_{i2}", tag="oc")
                        nc.vector.tensor_mul(och[:], o_ps[:], zb[:])
                        # chunk -> both batch-paired AllToAll slots
                        nc.sync.dma_start(a2a_in[h][i2], och[:])
                        nc.sync.dma_start(a2a_in[h][i2 + 4], och[:])
                    nc.gpsimd.collective_compute(
                        "AllToAll", mybir.AluOpType.bypass,
                        replica_groups=groups,
                        ins=[a2a_in[h].opt()], outs=[a2a_out[h].opt()])

            # ------------- phase 3: full-d output projection --------------
            with tc.tile_pool(name="sel3", bufs=1) as sel3, \
                 tc.tile_pool(name="blk", bufs=1) as blkp, \
                 tc.tile_pool(name="slot", bufs=4) as slotp, \
                 tc.tile_pool(name="w3", bufs=4) as w3, \
                 tc.tile_pool(name="st3", bufs=4) as st3, \
                 tc.tile_pool(name="ps3", bufs=8, space="PSUM") as ps3:
                sel_sb = sel3.tile([128, 2], F32)
                nc.sync.dma_start(sel_sb[:], sel_d.ap())
                # out^T d-blocks for this core's t-slice: select own batch's
                # slot pair (g, g+4) per source group g and head h
                blk = blkp.tile([128, KD * 4, 128], FR, name="blk")
                for g in range(HPG):
                    for h in range(HPG):
                        kd = g * HPG + h
                        s0 = slotp.tile([128, 512], FR, name=f"s0_{kd}", tag="s0")
                        nc.sync.dma_start(s0[:], a2a_out[h][g])
                        s1 = slotp.tile([128, 512], FR, name=f"s1_{kd}", tag="s1")
                        nc.sync.dma_start(s1[:], a2a_out[h][g + 4])
                        nc.vector.tensor_scalar_mul(s1[:], s1[:], sel_sb[:, 1:2])
                        nc.vector.scalar_tensor_tensor(
                            blk[:, kd * 4:(kd + 1) * 4, :]
                            .rearrange("p a b -> p (a b)"),
                            s0[:], sel_sb[:, 0:1], s1[:],
                            op0=mybir.AluOpType.mult, op1=mybir.AluOpType.add)
                for n in range(4):      # e blocks of 512
                    ys = []
                    for mi in range(4):
                        ps = ps3.tile([128, 512], F32, name=f"y{mi}_{n}", tag="ps")
                        ys.append(ps)
                    for kd in range(KD):
                        wt = w3.tile([128, 512], FR, name=f"w{kd}_{n}", tag="w")
                        nc.sync.dma_start(wt[:], wop_d.ap()[:, kd,
                                                            n * 512:(n + 1) * 512])
                        for mi in range(4):  # t chunks of 128 in our slice
                            nc.tensor.matmul(
                                ys[mi][:],
                                blk[:, kd * 4 + mi, :], wt[:],
                                start=(kd == 0), stop=(kd == KD - 1))
                    for mi in range(4):
                        st = st3.tile([128, 512], F32, name=f"ys{mi}_{n}", tag="st")
                        nc.vector.tensor_copy(st[:], ys[mi][:])
                        nc.sync.dma_start(
                            y_d.ap()[mi * 128:(mi + 1) * 128,
                                     n * 512:(n + 1) * 512], st[:])

    nc.compile()
    return nc


def _rope_tables():
    inv_freq = 1.0 / (ROPE_THETA ** (np.arange(0, DH, 2, dtype=np.float64) / DH))
    t = np.arange(T, dtype=np.float64)
    freqs = np.outer(t, inv_freq)                      # (T, 64)
    emb = np.concatenate([freqs, freqs], axis=-1)      # (T, DH)
    cosT = np.cos(emb).T.astype(np.float32).copy()     # (DH, T)
    sinT = np.sin(emb).T.astype(np.float32).copy()
    sinT[0:64, :] *= -1.0  # sign of -rotate-half folded into the table
    return cosT, sinT


def _masks():
    r = np.arange(128)[:, None]
    c = np.arange(512)[None, :]
    m = np.stack([(r <= c - 128 * i) for i in range(4)]).astype(np.float32)
    return np.ascontiguousarray(m.transpose(1, 0, 2))  # [128, 4, 512]


def _shard_inputs(x, w_qkv, w_op):
    cosT, sinT = _rope_tables()
    masks = _masks()
    ones = np.ones((128, 1), dtype=np.float32)
    # full w_op^T, arranged [p, kd, e] with d = kd*128 + p (same on all cores)
    wop = np.ascontiguousarray(w_op.T.reshape(KD, 128, D).transpose(1, 0, 2))
    in_maps = []
    for core in range(N_CORES):
        b, g = divmod(core, HPG)
        xt = np.ascontiguousarray(
            x[b].T.reshape(KD, 128, T).transpose(1, 0, 2))
        wq = w_qkv[g * 512:(g + 1) * 512]
        wk = w_qkv[D + g * 512:D + (g + 1) * 512]
        wqk = np.ascontiguousarray(
            np.concatenate([wq, wk], axis=0)
            .reshape(8, 128, KD, 128).transpose(3, 2, 0, 1))
        wv = np.ascontiguousarray(
            w_qkv[2 * D + g * 512:2 * D + (g + 1) * 512]
            .reshape(512, KD, 128).transpose(2, 1, 0))
        sel = np.zeros((128, 2), dtype=np.float32)
        sel[:, 0] = 1.0 if b == 0 else 0.0
        sel[:, 1] = 0.0 if b == 0 else 1.0
        in_maps.append({
            "xt": xt, "wqk": wqk, "wv": wv, "wop": wop,
            "cosT": cosT, "sinT": sinT, "masks": masks, "ones": ones,
            "sel": sel,
        })
    return in_maps


def kernel(x, w_qkv, w_op):
    x = np.asarray(x, dtype=np.float32)
    w_qkv = np.asarray(w_qkv, dtype=np.float32)
    w_op = np.asarray(w_op, dtype=np.float32)
    assert x.shape == (B, T, D) and w_qkv.shape == (3 * D, D) \
        and w_op.shape == (D, D)

    if "nc" not in _CACHE:
        _CACHE["nc"] = _build_program()
    nc = _CACHE["nc"]

    in_maps = _shard_inputs(x, w_qkv, w_op)
    trace = bool(int(os.environ.get("ATTN_KERNEL_TRACE", "0")))
    tmpdir = os.environ.get("ATTN_KERNEL_TRACE_DIR") or None
    res = run_bass_kernel_spmd(nc, in_maps, list(range(N_CORES)),
                               trace=trace, tmpdir=tmpdir)
    _CACHE["last_result"] = res

    # core 4b+g computed y rows [512g : 512(g+1)] of batch b
    y = np.empty((B, T, D), dtype=np.float32)
    for b in range(B):
        for g in range(HPG):
            y[b, 512 * g:512 * (g + 1)] = res.results[HPG * b + g]["y"]
    return y


# revision 18
# speedup vs baseline: 1.0139x; 1.0139x over previous
"""Trainium2 Bass kernel: causal multi-head attention with RoPE + output proj.

Reference computation (B=2, T=2048, D=2048, NH=16, DH=128, fp32):
    qkv = x @ w_qkv.T -> (B,T,3,NH,DH); rope(q,k); causal softmax attention;
    out = attn @ v; y = out @ w_op.T

Sharding over 8 NeuronCores: data parallel on batch (2) x tensor parallel on
heads (4 head-groups of 4 heads). Each core computes q/k/v for its 4 heads
and their attention output out^T (a 512-row d-block of out^T). A per-head
8-core AllToAll exchanges out^T t-slices so that core (b, g) ends up with the
FULL out^T for t-slice g of batch b; it then computes those 512 rows of the
final projection y = out @ w_op.T against the full w_op -- no inter-core
reduction needed. The host concatenates the 8 row-shards.

Batch routing through the single 8-core AllToAll is data-driven: each source
writes its chunk to both paired slots (r, r+4), and each destination combines
slot g and g+4 with a host-supplied 0/1 select (its own batch), keeping the
program SPMD-identical across cores.

All matmuls run as float32r (fp32 storage, FP22 multiply, full PE rate).
Scores are computed transposed (St[tk,tq] = k^T q) so the attn@v matmul needs
no on-chip transposes; the softmax denominator comes from a ones-vector
matmul accumulated alongside attn@v in PSUM.
"""

import os
import sys

import numpy as np

# The Bass program executes through jax/PJRT on the axon-tunneled NeuronCores.
# Guard against an environment that pinned jax to CPU before we import it.
if os.environ.get("TRN_TERMINAL_POOL_IPS") and "jax" not in sys.modules:
    if os.environ.get("JAX_PLATFORMS", "").strip() in ("cpu",):
        os.environ["JAX_PLATFORMS"] = ""

if "/opt/trn_rl_repo" not in sys.path and os.path.isdir("/opt/trn_rl_repo"):
    sys.path.append("/opt/trn_rl_repo")

import concourse.bacc as bacc
import concourse.mybir as mybir
import concourse.tile as tile
from concourse.bass_utils import run_bass_kernel_spmd

# Problem constants (hardcoded per contract).
B, T, D = 2, 2048, 2048
NH, DH = 16, 128
ROPE_THETA = 10000.0
N_CORES = 8
HPG = 4          # heads per core (head group size)
KD = D // 128    # 16 d-chunks of 128
TB = T // 512    # 4 t-blocks of 512
TC = T // 128    # 16 t-chunks of 128
SCALE = 1.0 / float(np.sqrt(DH))

F32 = mybir.dt.float32
FR = mybir.dt.float32r

_CACHE = {}


def _build_program():
    nc = bacc.Bacc("TRN2", target_bir_lowering=False, debug=False,
                   num_devices=N_CORES)

    # ---- external I/O (per-core shards; same program on all 8 cores) ----
    xt_d = nc.dram_tensor("xt", [128, KD, T], FR, kind="ExternalInput")
    wqk_d = nc.dram_tensor("wqk", [128, KD, 8, 128], FR, kind="ExternalInput")
    wv_d = nc.dram_tensor("wv", [128, KD, 512], FR, kind="ExternalInput")
    wop_d = nc.dram_tensor("wop", [128, KD, D], FR, kind="ExternalInput")
    cos_d = nc.dram_tensor("cosT", [128, T], F32, kind="ExternalInput")
    sin_d = nc.dram_tensor("sinT", [128, T], F32, kind="ExternalInput")
    mask_d = nc.dram_tensor("masks", [128, 4, 512], F32, kind="ExternalInput")
    ones_d = nc.dram_tensor("ones", [128, 128], FR, kind="ExternalInput")
    sel_d = nc.dram_tensor("sel", [128, 2], F32, kind="ExternalInput")
    y_d = nc.dram_tensor("y", [512, D], F32, kind="ExternalOutput")

    groups = [[0, 1, 2, 3, 4, 5, 6, 7]]

    with tile.TileContext(nc) as tc:
        with tc.tile_pool(name="dram", bufs=1, space="DRAM") as dram:
            qk_spill = dram.tile([2, HPG, 128, T], F32)   # q/k pre-RoPE
            v_spill = dram.tile([TC, 128, 512], FR)

            # ------------- phase 1: q/k/v projections (single pass) -------
            with tc.tile_pool(name="w1", bufs=1) as w1, \
                 tc.tile_pool(name="xs", bufs=2) as xs, \
                 tc.tile_pool(name="st1", bufs=3) as st1, \
                 tc.tile_pool(name="ps1", bufs=4, space="PSUM") as ps1:
                wqk_sb = w1.tile([128, KD, 8, 128], FR, name="wqk_sb")
                nc.sync.dma_start(wqk_sb[:], wqk_d.ap())
                wv_sb = w1.tile([128, KD, 512], FR, name="wv_sb")
                nc.sync.dma_start(wv_sb[:], wv_d.ap())
                for tb in range(TB):
                    xt = xs.tile([128, KD, 512], FR, name=f"xt{tb}", tag="xt")
                    nc.sync.dma_start(xt[:], xt_d.ap()[:, :, tb * 512:(tb + 1) * 512])
                    # interleave q/k heads so head 0's spill completes early
                    for j in (0, 4, 1, 5, 2, 6, 3, 7):
                        ps = ps1.tile([128, 512], F32, name=f"psqk{tb}_{j}", tag="ps")
                        for kd in range(KD):
                            nc.tensor.matmul(ps[:], wqk_sb[:, kd, j, :], xt[:, kd, :],
                                             start=(kd == 0), stop=(kd == KD - 1))
                        st = st1.tile([128, 512], F32, name=f"stqk{tb}_{j}", tag="st")
                        nc.scalar.copy(st[:], ps[:])
                        nc.sync.dma_start(
                            qk_spill[j // HPG, j % HPG, :, tb * 512:(tb + 1) * 512],
                            st[:])
                    for i in range(4):  # v t-chunks
                        tci = tb * 4 + i
                        ps = ps1.tile([128, 512], F32, name=f"psv{tb}_{i}", tag="ps")
                        for kd in range(KD):
                            nc.tensor.matmul(ps[:], xt[:, kd, i * 128:(i + 1) * 128],
                                             wv_sb[:, kd, :],
                                             start=(kd == 0), stop=(kd == KD - 1))
                        stv = st1.tile([128, 512], FR, name=f"stv{tb}_{i}", tag="stv")
                        nc.scalar.copy(stv[:], ps[:])
                        nc.sync.dma_start(v_spill[tci], stv[:])

            # ------------- phase 2: attention + per-head AllToAll ---------
            a2a_in = [dram.tile([8, 128, 512], FR, name=f"a2ain{h}", tag=f"a2ain{h}")
                      for h in range(HPG)]
            a2a_out = [dram.tile([8, 128, 512], FR, name=f"a2aout{h}",
                                 tag=f"a2aout{h}") for h in range(HPG)]
            with tc.tile_pool(name="vres", bufs=1) as vres, \
                 tc.tile_pool(name="consts", bufs=1) as consts, \
                 tc.tile_pool(name="qk", bufs=2) as qkp, \
                 tc.tile_pool(name="rot", bufs=2) as rotp, \
                 tc.tile_pool(name="ropeqk", bufs=2) as ropep, \
                 tc.tile_pool(name="ptile", bufs=6) as ptp, \
                 tc.tile_pool(name="zt", bufs=4) as ztp, \
                 tc.tile_pool(name="och", bufs=3) as ochp, \
                 tc.tile_pool(name="sps", bufs=2, space="PSUM") as sps, \
                 tc.tile_pool(name="ops", bufs=4, space="PSUM") as ops, \
                 tc.tile_pool(name="zps", bufs=2, space="PSUM") as zps:
                v_sb = vres.tile([128, TC, 512], FR)
                for tci in range(TC):  # chunked so each reload fires early
                    nc.sync.dma_start(v_sb[:, tci, :], v_spill[tci])
                cos_sb = consts.tile([128, T], F32)
                sin_sb = consts.tile([128, T], F32)
                mask_sb = consts.tile([128, 4, 512], F32)
                ones_sb = consts.tile([128, 128], FR)
                nc.sync.dma_start(cos_sb[:], cos_d.ap())
                nc.sync.dma_start(sin_sb[:], sin_d.ap())
                nc.sync.dma_start(mask_sb[:], mask_d.ap())
                nc.sync.dma_start(ones_sb[:], ones_d.ap())

                def rope(h):
                    qraw = qkp.tile([128, T], F32, name=f"qraw{h}", tag="qraw")
                    nc.sync.dma_start(qraw[:], qk_spill[0, h])
                    kraw = qkp.tile([128, T], F32, name=f"kraw{h}", tag="kraw")
                    nc.sync.dma_start(kraw[:], qk_spill[1, h])
                    # rotate_half via partition-swapped reload (DVE ops cannot
                    # cross partitions); the sign lives in the sin table.
                    qrot = rotp.tile([128, T], F32, name=f"qrot{h}", tag="qrot")
                    nc.sync.dma_start(qrot[0:64, :], qk_spill[0, h, 64:128, :])
                    nc.sync.dma_start(qrot[64:128, :], qk_spill[0, h, 0:64, :])
                    krot = rotp.tile([128, T], F32, name=f"krot{h}", tag="krot")
                    nc.sync.dma_start(krot[0:64, :], qk_spill[1, h, 64:128, :])
                    nc.sync.dma_start(krot[64:128, :], qk_spill[1, h, 0:64, :])
                    # RoPE: out = a*cos + rotate_half(a)*sin_signed
                    qr = ropep.tile([128, T], FR, name=f"qr{h}", tag="qr")
                    kr = ropep.tile([128, T], FR, name=f"kr{h}", tag="kr")
                    for src, rot, dst in ((qraw, qrot, qr), (kraw, krot, kr)):
                        nc.vector.tensor_mul(src[:], src[:], cos_sb[:])
                        nc.vector.tensor_mul(rot[:], rot[:], sin_sb[:])
                        nc.vector.tensor_add(dst[:], src[:], rot[:])
                    return qr, kr

                qr, kr = rope(0)
                for h in range(HPG):
                    o_banks, zb_tiles = [], []
                    for i2 in range(4):  # tq blocks of 512
                        n_j = 4 * i2 + 4
                        o_ps = ops.tile([128, 512], F32, name=f"o{h}_{i2}", tag="o")
                        z_ps = zps.tile([128, 512], F32, name=f"z{h}_{i2}", tag="z")
                        for j2 in range(n_j):  # tk chunks of 128
                            s_ps = sps.tile([128, 512], F32,
                                            name=f"s{h}_{i2}_{j2}", tag="s")
                            nc.tensor.matmul(s_ps[:], kr[:, j2 * 128:(j2 + 1) * 128],
                                             qr[:, i2 * 512:(i2 + 1) * 512],
                                             start=True, stop=True)
                            pt = ptp.tile([128, 512], FR,
                                          name=f"p{h}_{i2}_{j2}", tag="p")
                            nc.scalar.activation(
                                pt[:], s_ps[:],
                                mybir.ActivationFunctionType.Exp, scale=SCALE)
                            m = j2 - 4 * i2
                            if m >= 0:  # partial (diagonal) tile: apply mask
                                nc.vector.tensor_mul(pt[:], pt[:], mask_sb[:, m, :])
                            nc.tensor.matmul(o_ps[:], v_sb[:, j2, h * 128:(h + 1) * 128],
                                             pt[:], start=(j2 == 0),
                                             stop=(j2 == n_j - 1))
                            # ones-matrix lhsT -> every row of z_ps holds the
                            # softmax denominator (pre-broadcast)
                            nc.tensor.matmul(z_ps[:], ones_sb[:], pt[:],
                                             start=(j2 == 0), stop=(j2 == n_j - 1))
                        zb = ztp.tile([128, 512], F32, name=f"zb{h}_{i2}", tag="zb")
                        nc.vector.reciprocal_approx_fast(zb[:], z_ps[:])
                        o_banks.append(o_ps)
                        zb_tiles.append(zb)
                    # next head's RoPE goes ahead of this head's epilogue so
                    # its scores aren't blocked behind the DVE queue
                    if h + 1 < HPG:
                        qr, kr = rope(h + 1)
                    for i2 in range(4):
                        och = ochp.tile([128, 512], FR, name=f"oc{h}_{i2}", tag="oc")
                        nc.vector.tensor_mul(och[:], o_banks[i2][:], zb_tiles[i2][:])
                        # chunk -> both batch-paired AllToAll slots
                        nc.sync.dma_start(a2a_in[h][i2], och[:])
                        nc.sync.dma_start(a2a_in[h][i2 + 4], och[:])
                    nc.gpsimd.collective_compute(
                        "AllToAll", mybir.AluOpType.bypass,
                        replica_groups=groups,
                        ins=[a2a_in[h].opt()], outs=[a2a_out[h].opt()])

            # ------------- phase 3: full-d output projection --------------
            with tc.tile_pool(name="sel3", bufs=1) as sel3, \
                 tc.tile_pool(name="blk", bufs=1) as blkp, \
                 tc.tile_pool(name="slot", bufs=4) as slotp, \
                 tc.tile_pool(name="w3", bufs=4) as w3, \
                 tc.tile_pool(name="st3", bufs=4) as st3, \
                 tc.tile_pool(name="ps3", bufs=8, space="PSUM") as ps3:
                sel_sb = sel3.tile([128, 2], F32)
                nc.sync.dma_start(sel_sb[:], sel_d.ap())
                # out^T d-blocks for this core's t-slice: select own batch's
                # slot pair (g, g+4) per source group g and head h
                blk = blkp.tile([128, KD * 4, 128], FR, name="blk")
                for g in range(HPG):
                    for h in range(HPG):
                        kd = g * HPG + h
                        s0 = slotp.tile([128, 512], FR, name=f"s0_{kd}", tag="s0")
                        nc.sync.dma_start(s0[:], a2a_out[h][g])
                        s1 = slotp.tile([128, 512], FR, name=f"s1_{kd}", tag="s1")
                        nc.sync.dma_start(s1[:], a2a_out[h][g + 4])
                        nc.vector.tensor_scalar_mul(s1[:], s1[:], sel_sb[:, 1:2])
                        nc.vector.scalar_tensor_tensor(
                            blk[:, kd * 4:(kd + 1) * 4, :]
                            .rearrange("p a b -> p (a b)"),
                            s0[:], sel_sb[:, 0:1], s1[:],
                            op0=mybir.AluOpType.mult, op1=mybir.AluOpType.add)
                for n in range(4):      # e blocks of 512
                    ys = []
                    for mi in range(4):
                        ps = ps3.tile([128, 512], F32, name=f"y{mi}_{n}", tag="ps")
                        ys.append(ps)
                    # accumulate head-major so each e-block's first 3/4 of the
                    # contraction can run before the last AllToAll lands
                    for hi, (h, g) in enumerate(
                            (h, g) for h in range(HPG) for g in range(HPG)):
                        kd = g * HPG + h
                        wt = w3.tile([128, 512], FR, name=f"w{kd}_{n}", tag="w")
                        nc.sync.dma_start(wt[:], wop_d.ap()[:, kd,
                                                            n * 512:(n + 1) * 512])
                        for mi in range(4):  # t chunks of 128 in our slice
                            nc.tensor.matmul(
                                ys[mi][:],
                                blk[:, kd * 4 + mi, :], wt[:],
                                start=(hi == 0), stop=(hi == KD - 1))
                    for mi in range(4):
                        st = st3.tile([128, 512], F32, name=f"ys{mi}_{n}", tag="st")
                        nc.vector.tensor_copy(st[:], ys[mi][:])
                        nc.sync.dma_start(
                            y_d.ap()[mi * 128:(mi + 1) * 128,
                                     n * 512:(n + 1) * 512], st[:])

    nc.compile()
    return nc


def _rope_tables():
    inv_freq = 1.0 / (ROPE_THETA ** (np.arange(0, DH, 2, dtype=np.float64) / DH))
    t = np.arange(T, dtype=np.float64)
    freqs = np.outer(t, inv_freq)                      # (T, 64)
    emb = np.concatenate([freqs, freqs], axis=-1)      # (T, DH)
    cosT = np.cos(emb).T.astype(np.float32).copy()     # (DH, T)
    sinT = np.sin(emb).T.astype(np.float32).copy()
    sinT[0:64, :] *= -1.0  # sign of -rotate-half folded into the table
    return cosT, sinT


def _masks():
    r = np.arange(128)[:, None]
    c = np.arange(512)[None, :]
    m = np.stack([(r <= c - 128 * i) for i in range(4)]).astype(np.float32)
    return np.ascontiguousarray(m.transpose(1, 0, 2))  # [128, 4, 512]


def _shard_inputs(x, w_qkv, w_op):
    cosT, sinT = _rope_tables()
    masks = _masks()
    ones = np.ones((128, 128), dtype=np.float32)
    # full w_op^T, arranged [p, kd, e] with d = kd*128 + p (same on all cores)
    wop = np.ascontiguousarray(w_op.T.reshape(KD, 128, D).transpose(1, 0, 2))
    in_maps = []
    for core in range(N_CORES):
        b, g = divmod(core, HPG)
        xt = np.ascontiguousarray(
            x[b].T.reshape(KD, 128, T).transpose(1, 0, 2))
        wq = w_qkv[g * 512:(g + 1) * 512]
        wk = w_qkv[D + g * 512:D + (g + 1) * 512]
        wqk = np.ascontiguousarray(
            np.concatenate([wq, wk], axis=0)
            .reshape(8, 128, KD, 128).transpose(3, 2, 0, 1))
        wv = np.ascontiguousarray(
            w_qkv[2 * D + g * 512:2 * D + (g + 1) * 512]
            .reshape(512, KD, 128).transpose(2, 1, 0))
        sel = np.zeros((128, 2), dtype=np.float32)
        sel[:, 0] = 1.0 if b == 0 else 0.0
        sel[:, 1] = 0.0 if b == 0 else 1.0
        in_maps.append({
            "xt": xt, "wqk": wqk, "wv": wv, "wop": wop,
            "cosT": cosT, "sinT": sinT, "masks": masks, "ones": ones,
            "sel": sel,
        })
    return in_maps


def kernel(x, w_qkv, w_op):
    x = np.asarray(x, dtype=np.float32)
    w_qkv = np.asarray(w_qkv, dtype=np.float32)
    w_op = np.asarray(w_op, dtype=np.float32)
    assert x.shape == (B, T, D) and w_qkv.shape == (3 * D, D) \
        and w_op.shape == (D, D)

    if "nc" not in _CACHE:
        _CACHE["nc"] = _build_program()
    nc = _CACHE["nc"]

    in_maps = _shard_inputs(x, w_qkv, w_op)
    trace = bool(int(os.environ.get("ATTN_KERNEL_TRACE", "0")))
    tmpdir = os.environ.get("ATTN_KERNEL_TRACE_DIR") or None
    res = run_bass_kernel_spmd(nc, in_maps, list(range(N_CORES)),
                               trace=trace, tmpdir=tmpdir)
    _CACHE["last_result"] = res

    # core 4b+g computed y rows [512g : 512(g+1)] of batch b
    y = np.empty((B, T, D), dtype=np.float32)
    for b in range(B):
        for g in range(HPG):
            y[b, 512 * g:512 * (g + 1)] = res.results[HPG * b + g]["y"]
    return y
